# revision 2
# baseline (speedup 1.0000x reference)
"""Trainium2 Bass kernel for nn_ConditionalMLN.

Math: the reference reduces exactly (cart.sum(-1) == 1 algebraically) to
    out = sum_r w_r * (G + cnt_r - S_r),   S_r = sum_g flag[r,g] * Z[r,g]
    Z = prod_k t_k,  t_k = select(mask_k, p[i_k], 1 - p[i_k])
        = sigma_k * (p[i_k] + m_k - 1),  sigma_k = 2*m_k - 1
so each NeuronCore computes S_r for its 2 rules (R=16 sharded over 8 cores)
via 1.2M table gathers + elementwise products + a reduction.

Gather: per-element indirect DMA (SWDGE row-mode: 128 offsets -> 128 scalar
descriptors per instruction), which is the only per-element gather primitive
that compiles and runs correctly on this toolchain.
"""

import numpy as np

R, G, K, N = 16, 200000, 3, 2000000
NCORES = 8
P = 128
RLOC = R // NCORES            # rules per core
GCOLS = (G + P - 1) // P      # 1563 columns per rule (G padded to 200064)
GPAD = GCOLS * P
COLS = RLOC * GCOLS           # 3126 columns per core

_CACHE = {}


def _build_program():
    from concourse import bass, mybir

    nc = bass.Bass("TRN2", target_bir_lowering=False, debug=False,
                   num_devices=NCORES)

    table = nc.declare_dram_parameter("table", [N, 1], mybir.dt.float32,
                                      isOutput=False)
    idx_d = [nc.declare_dram_parameter(f"idx{k}", [P, COLS], mybir.dt.int32,
                                       isOutput=False) for k in range(K)]
    msk_d = [nc.declare_dram_parameter(f"msk{k}", [P, COLS], mybir.dt.int8,
                                       isOutput=False) for k in range(K)]
    flg_d = nc.declare_dram_parameter("flg", [P, COLS], mybir.dt.int8,
                                      isOutput=False)
    y_d = nc.declare_dram_parameter("y", [P, RLOC], mybir.dt.float32,
                                    isOutput=True)

    f32, i32, i8 = mybir.dt.float32, mybir.dt.int32, mybir.dt.int8
    idx_s = [nc.alloc_sbuf_tensor(f"idx{k}_s", [P, COLS], i32) for k in range(K)]
    msk_s = [nc.alloc_sbuf_tensor(f"msk{k}_s", [P, COLS], i8) for k in range(K)]
    flg_s = nc.alloc_sbuf_tensor("flg_s", [P, COLS], i8)
    p_s = [nc.alloc_sbuf_tensor(f"p{k}_s", [P, COLS], f32) for k in range(K)]
    mf_s = nc.alloc_sbuf_tensor("mf_s", [P, COLS], f32)
    sg_s = nc.alloc_sbuf_tensor("sg_s", [P, COLS], f32)
    z_s = nc.alloc_sbuf_tensor("z_s", [P, COLS], f32)
    acc_s = nc.alloc_sbuf_tensor("acc_s", [P, RLOC], f32)

    NDMA_IN = 2 * K + 1
    AluOp = mybir.AluOpType

    with (
        nc.Block() as block,
        nc.semaphore("dsem") as dsem,
        nc.semaphore("gsem") as gsem,
        nc.semaphore("vsem") as vsem,
        nc.semaphore("osem") as osem,
    ):

        @block.sync
        def _(sync):
            for k in range(K):
                sync.dma_start(out=idx_s[k].ap(), in_=idx_d[k][:]).then_inc(dsem, 16)
            for k in range(K):
                sync.dma_start(out=msk_s[k].ap(), in_=msk_d[k][:]).then_inc(dsem, 16)
            sync.dma_start(out=flg_s.ap(), in_=flg_d[:]).then_inc(dsem, 16)
            sync.wait_ge(vsem, 1)
            sync.dma_start(out=y_d[:], in_=acc_s.ap()).then_inc(osem, 16)
            sync.wait_ge(osem, 16)

        @block.gpsimd
        def _(g):
            g.wait_ge(dsem, 16 * K)  # idx planes resident
            for k in range(K):
                ip = idx_s[k].ap()
                op = p_s[k].ap()
                for j in range(COLS):
                    g.indirect_dma_start(
                        out=op[:, j:j + 1],
                        out_offset=None,
                        in_=table[:],
                        in_offset=bass.IndirectOffsetOnAxis(
                            ap=ip[:, j:j + 1], axis=0),
                    ).then_inc(gsem, 16)

        @block.vector
        def _(v):
            v.wait_ge(dsem, 16 * NDMA_IN)
            v.wait_ge(gsem, 16 * K * COLS)
            mf = mf_s.ap()
            sg = sg_s.ap()
            z = z_s.ap()
            for k in range(K):
                p = p_s[k].ap()
                v.tensor_copy(mf, msk_s[k].ap())            # int8 -> f32
                # d_k = p + m - 1   (stored in p plane)
                v.tensor_tensor(out=p, in0=p, in1=mf, op=AluOp.add)
                v.tensor_scalar(out=p, in0=p, scalar1=-1.0, scalar2=None,
                                op0=AluOp.add)
                # sigma_k = 2m - 1 ; accumulate product of sigmas in sg
                v.tensor_scalar(out=mf, in0=mf, scalar1=2.0, scalar2=-1.0,
                                op0=AluOp.mult, op1=AluOp.add)
                if k == 0:
                    v.tensor_copy(sg, mf)
                else:
                    v.tensor_tensor(out=sg, in0=sg, in1=mf, op=AluOp.mult)
            # z = d0*d1*d2 * sg * flag
            v.tensor_tensor(out=z, in0=p_s[0].ap(), in1=p_s[1].ap(), op=AluOp.mult)
            v.tensor_tensor(out=z, in0=z, in1=p_s[2].ap(), op=AluOp.mult)
            v.tensor_tensor(out=z, in0=z, in1=sg, op=AluOp.mult)
            v.tensor_copy(mf, flg_s.ap())                   # int8 -> f32
            v.tensor_tensor(out=z, in0=z, in1=mf, op=AluOp.mult)
            for r in range(RLOC):
                red = v.tensor_reduce(
                    acc_s.ap()[:, r:r + 1],
                    z[:, r * GCOLS:(r + 1) * GCOLS],
                    mybir.AxisListType.X,
                    AluOp.add,
                )
            red.then_inc(vsem, 1)

    return nc


def _layout(a):
    """[RLOC, G] -> [P, COLS] with element (r, g) at [g % P, r*GCOLS + g//P]."""
    rloc = a.shape[0]
    pad = np.zeros((rloc, GPAD - G), dtype=a.dtype)
    ap = np.concatenate([a, pad], axis=1)          # [RLOC, GPAD]
    ap = ap.reshape(rloc, GCOLS, P)                # [r, col, part]
    ap = np.transpose(ap, (2, 0, 1)).reshape(P, rloc * GCOLS)
    return np.ascontiguousarray(ap)


def kernel(posterior_prob, observed_rule_cnts, rule_weights,
           latent_var_inds, latent_neg_mask, obs_zero_flag):
    posterior_prob = np.asarray(posterior_prob)
    observed_rule_cnts = np.asarray(observed_rule_cnts)
    rule_weights = np.asarray(rule_weights)
    latent_var_inds = np.asarray(latent_var_inds)
    latent_neg_mask = np.asarray(latent_neg_mask)
    obs_zero_flag = np.asarray(obs_zero_flag)

    if "nc" not in _CACHE:
        _CACHE["nc"] = _build_program()
    nc = _CACHE["nc"]

    table = np.ascontiguousarray(posterior_prob.astype(np.float32).reshape(N, 1))
    in_maps = []
    for c in range(NCORES):
        rules = slice(RLOC * c, RLOC * (c + 1))
        m = {"table": table, "flg": _layout(
            obs_zero_flag[rules].astype(np.int8))}
        for k in range(K):
            m[f"idx{k}"] = _layout(
                latent_var_inds[rules, :, k].astype(np.int32))
            m[f"msk{k}"] = _layout(
                latent_neg_mask[rules, :, k].astype(np.int8))
        in_maps.append(m)

    from concourse.bass_utils import run_bass_kernel_spmd
    res = run_bass_kernel_spmd(nc, in_maps, core_ids=list(range(NCORES)))

    s = np.empty(R, dtype=np.float64)
    for c in range(NCORES):
        part = res.results[c]["y"].sum(axis=0)      # [RLOC]
        s[RLOC * c:RLOC * (c + 1)] = part
    scores = np.float64(G) + observed_rule_cnts.astype(np.float64) - s
    out = rule_weights.astype(np.float64) @ scores
    return np.asarray([out], dtype=np.float32)
